# revision 22
# baseline (speedup 1.0000x reference)
"""Gaussian row-smoothing (sigma=h_smooth, truncate=4.0, reflect padding) on
8 Trainium2 NeuronCores.

Strategy
--------
Data-parallel over rows (nz=4096 -> 512 rows/core). The 1D conv along rows is
computed on the TensorEngine as a banded-Toeplitz matmul in the transposed
domain:

  host: per core, pad the [512, 8192] shard symmetrically by r=40 along cols,
        transpose to [8272, 512], cast to bf16 and lay out as 17 superblocks
        of 4 column-tiles [128, 512] each (tiles 64..67 mostly zero-padding;
        only tile 64 is ever transferred).

  device: output column-block b (128 cols x 512 rows, transposed layout) is
        psum_b = WA.T @ tile_b + WB.T @ tile_{b+1}
        where WA[p, j] = w[p - j]       (0 <= p-j <= 2r)
              WB[p, j] = w[128 + p - j] (0 <= 128+p-j <= 2r)
        are constant [128, 128] bf16 band matrices holding the 81-tap kernel.
        PSUM (f32) -> SBUF copy casts to bf16 (alternating DVE / ACT so
        neither engine becomes the bottleneck), output DMA'd as bf16.

  host: un-interleave, cast back to f32, transpose, concatenate.

All HBM traffic is bf16 (16.9 MB/core vs 33 MB for f32), which matters
because the kernel is HBM-bound: floor = 16.9 MB / 358 GB/s ~ 47 us.
Input loads ride the sync (SP) HWDGE ring as 512 KB fully-contiguous DMAs;
output stores ride the scalar (ACT) ring so their data dependencies never
stall input streaming. bf16 quantization of input/weights/output gives
l2 rel err ~3e-3 (tolerance 2e-2).
"""

import os
import numpy as np

NZ, NX = 4096, 8192
N_CORES = 8
RPC = NZ // N_CORES          # rows per core = 512
BLK = 128                    # column block (partition dim)
NCH = NX // BLK              # 64 output column blocks per row
NT = NCH + 1                 # 65 input tiles (one extra for the right overlap)
TPS = 4                      # tiles per input superblock (512 KB DMAs)
NSB = NT // TPS              # 16 full superblocks; tile 64 rides separately
GPO = 2                      # psum groups per output tile (4 blocks, 512 KB DMAs)
TRUNCATE = 4.0
# fp8o: e3m4 in+out; fp8: e3m4 in, bf16 out; bf16: bf16 in+out
MODE = os.environ.get("KERNEL_MODE", "fp8o")
OUT_SCALE = 16.0  # folded into the weights when the output is e3m4
X_BUFS = int(os.environ.get("KERNEL_XBUFS", "17"))
O_BUFS = int(os.environ.get("KERNEL_OBUFS", "4"))
COPY_SPLIT = os.environ.get("KERNEL_COPY_SPLIT", "1") == "1"
# Junk matmuls bridging the DMA-only prologue so the PE HAM clock gate
# (1.2 -> 2.4 GHz after ~3.4us sustained activity) lifts before real work.
N_WARMUP = int(os.environ.get("KERNEL_WARMUP", "12"))
OUT_ENG = os.environ.get("KERNEL_OUT_ENG", "sync")
PREFETCH = os.environ.get("KERNEL_PREFETCH", "1") == "1"
W_ENG = os.environ.get("KERNEL_W_ENG", "sync")
SPLIT_LAST = os.environ.get("KERNEL_SPLIT_LAST", "1") == "1"


def OUT_DMA_ENG(nc):
    return {"gpsimd": nc.gpsimd, "scalar": nc.scalar, "sync": nc.sync}[OUT_ENG]

_NC_CACHE = {}


def _gauss_weights(sigma: float) -> tuple[np.ndarray, int]:
    radius = int(TRUNCATE * sigma + 0.5)
    x = np.arange(-radius, radius + 1, dtype=np.float32)
    w = np.exp(np.float32(-0.5) * (x / np.float32(sigma)) ** 2)
    w = w / np.sum(w)
    return w.astype(np.float32), radius


def _band_matrices(sigma: float) -> tuple[np.ndarray, np.ndarray, int]:
    w, r = _gauss_weights(sigma)
    ntaps = 2 * r + 1
    assert ntaps <= BLK, f"kernel supports radius <= 63, got {r}"
    wa = np.zeros((BLK, BLK), np.float32)
    wb = np.zeros((BLK, BLK), np.float32)
    p = np.arange(BLK)[:, None]
    j = np.arange(BLK)[None, :]
    k = p - j
    m = (k >= 0) & (k <= 2 * r)
    wa[m] = w[k[m]]
    k2 = k + BLK
    m2 = (k2 >= 0) & (k2 <= 2 * r)
    wb[m2] = w[k2[m2]]
    return wa, wb, r


def build_nc():
    """Build (and cache) the SPMD Bass program. Shapes are fixed; the band
    weights arrive as data, so one NEFF serves any h_smooth with radius<=63."""
    if "nc" in _NC_CACHE:
        return _NC_CACHE["nc"]
    import concourse.tile as tile
    from concourse import bacc, mybir

    f32 = mybir.dt.float32
    bf16 = mybir.dt.bfloat16
    xdt = bf16 if MODE == "bf16" else mybir.dt.float8e3
    odt = mybir.dt.float8e3 if MODE == "fp8o" else bf16

    nc = bacc.Bacc(None)
    # 17 superblocks of 4 tiles; last superblock only has tile 64 valid.
    xt = nc.declare_dram_parameter("xt", [(NSB + 1) * BLK, TPS * RPC], xdt,
                                   isOutput=False)
    wa_p = nc.declare_dram_parameter("wa", [BLK, BLK], bf16, isOutput=False)
    wb_p = nc.declare_dram_parameter("wb", [BLK, BLK], bf16, isOutput=False)
    # Output: 16 groups of 4 column-blocks, each [128, 4*512] contiguous.
    out = nc.declare_dram_parameter("out", [(NCH // 4) * BLK, 4 * RPC], odt,
                                    isOutput=True)

    with tile.TileContext(nc) as tc:
        with (
            tc.tile_pool(name="w", bufs=1) as wpool,
            tc.tile_pool(name="x", bufs=X_BUFS) as xpool,
            tc.tile_pool(name="xtl", bufs=1) as xtlpool,
            tc.tile_pool(name="ps", bufs=4, space="PSUM") as pspool,
            tc.tile_pool(name="o", bufs=O_BUFS) as opool,
        ):
            weng = OUT_DMA_ENG(nc) if W_ENG == "out" else {
                "gpsimd": nc.gpsimd, "scalar": nc.scalar, "sync": nc.sync
            }[W_ENG]
            wa_t = wpool.tile([BLK, BLK], bf16, tag="wa")
            weng.dma_start(wa_t[:], wa_p[:])
            wb_t = wpool.tile([BLK, BLK], bf16, tag="wb")
            weng.dma_start(wb_t[:], wb_p[:])

            if N_WARMUP:
                wu = pspool.tile([BLK, 2 * RPC], f32, tag="psum")
                for _ in range(N_WARMUP):
                    nc.tensor.matmul(
                        wu[:, 0:BLK], wa_t[:], wa_t[:], start=True, stop=True
                    )

            sb_bufs = {}

            def ensure_loaded(s):
                if s in sb_bufs:
                    return
                if s < NSB:
                    tl = xpool.tile([BLK, TPS * RPC], xdt, tag="xsb")
                    nc.sync.dma_start(tl[:], xt[s * BLK : (s + 1) * BLK, :])
                else:  # tail: only tile 64 (first slot of superblock 16)
                    tl = xtlpool.tile([BLK, RPC], xdt, tag="xtail")
                    nc.sync.dma_start(tl[:], xt[NSB * BLK : (NSB + 1) * BLK, 0:RPC])
                sb_bufs[s] = tl

            def tile_ap(t):
                s, slot = t // TPS, t % TPS
                ensure_loaded(s)
                if s < NSB:
                    return sb_bufs[s][:, slot * RPC : (slot + 1) * RPC]
                return sb_bufs[s][:]

            otile_box = [None]

            def emit_output(g, ps):
                h = g % GPO
                if h == 0:
                    otile_box[0] = opool.tile(
                        [BLK, GPO * 2 * RPC], odt, tag="otile",
                        name=f"ot{g // GPO}",
                    )
                ot = otile_box[0]
                dst = ot[:, h * 2 * RPC : (h + 1) * 2 * RPC]
                if COPY_SPLIT:
                    # halve group-copy latency: DVE takes bank A, ACT bank B
                    nc.vector.tensor_copy(dst[:, 0:RPC], ps[:, 0:RPC])
                    nc.scalar.copy(dst[:, RPC:], ps[:, RPC:])
                else:
                    nc.vector.tensor_copy(dst, ps[:])
                if h == GPO - 1:
                    g4 = g // GPO
                    if SPLIT_LAST and g == NCH // 2 - 1:
                        # last otile: ship each half right after its copy so
                        # the final chunk doesn't wait for both copies
                        OUT_DMA_ENG(nc).dma_start(
                            out[g4 * BLK : (g4 + 1) * BLK, 0 : 2 * RPC],
                            ot[:, 0 : 2 * RPC],
                        )
                        OUT_DMA_ENG(nc).dma_start(
                            out[g4 * BLK : (g4 + 1) * BLK, 2 * RPC :],
                            ot[:, 2 * RPC :],
                        )
                    else:
                        OUT_DMA_ENG(nc).dma_start(
                            out[g4 * BLK : (g4 + 1) * BLK, :], ot[:]
                        )

            if PREFETCH:
                # queue every input DMA ahead of any output DMA so an
                # output's data-dependency stall can never delay an input
                # when both share the sync HWDGE ring
                assert X_BUFS >= NSB + 1
                for s in range(NSB + 1):
                    ensure_loaded(s)

            WAVE = 4  # psum groups per wave (all 8 PSUM banks)
            for wv in range(NCH // 2 // WAVE):
                gs = [WAVE * wv + i for i in range(WAVE)]
                pss = [
                    pspool.tile([BLK, 2 * RPC], f32, tag="psum", name=f"ps{g}")
                    for g in gs
                ]
                # one LDWEIGHTS per pass instead of per matmul: all wa
                # accumulations for the wave, then all wb
                for i, g in enumerate(gs):
                    nc.tensor.matmul(pss[i][:, 0:RPC], wa_t[:], tile_ap(2 * g),
                                     start=True, stop=False)
                    nc.tensor.matmul(pss[i][:, RPC:], wa_t[:], tile_ap(2 * g + 1),
                                     start=True, stop=False)
                for i, g in enumerate(gs):
                    nc.tensor.matmul(pss[i][:, 0:RPC], wb_t[:], tile_ap(2 * g + 1),
                                     start=False, stop=True)
                    nc.tensor.matmul(pss[i][:, RPC:], wb_t[:], tile_ap(2 * g + 2),
                                     start=False, stop=True)
                for i, g in enumerate(gs):
                    emit_output(g, pss[i])

    nc.finalize()
    _NC_CACHE["nc"] = nc
    return nc


def _shaped_quant_e3m4(a: np.ndarray):
    """Cast rows to float8_e3m4 with first-order error feedback along the row.
    The Gaussian filter is a strong low-pass, so pushing quantization noise
    to high frequencies makes it vanish from the output (~14x less noise
    than round-to-nearest while sending the identical byte count)."""
    import ml_dtypes

    q = np.empty(a.shape, ml_dtypes.float8_e3m4)
    e = np.zeros(a.shape[0], np.float32)
    for j in range(a.shape[1]):
        v = a[:, j] + e
        qj = v.astype(ml_dtypes.float8_e3m4)
        q[:, j] = qj
        e = v - qj.astype(np.float32)
    return q


def make_in_maps(feature: np.ndarray, h_smooth) -> list[dict]:
    import ml_dtypes

    sigma = float(int(h_smooth))
    wa, wb, r = _band_matrices(sigma)
    ws = np.float32(OUT_SCALE) if MODE == "fp8o" else np.float32(1.0)
    wmap = {
        "wa": (wa * ws).astype(ml_dtypes.bfloat16),
        "wb": (wb * ws).astype(ml_dtypes.bfloat16),
    }
    feature = np.asarray(feature, dtype=np.float32)
    assert feature.shape == (NZ, NX)
    xp_full = np.pad(feature, ((0, 0), (r, r)), mode="symmetric")  # [nz, nx+2r]
    if MODE != "bf16":
        xq_full = _shaped_quant_e3m4(xp_full)
        xcast = ml_dtypes.float8_e3m4
    else:
        xq_full = xp_full.astype(ml_dtypes.bfloat16)
        xcast = ml_dtypes.bfloat16
    in_maps = []
    for c in range(N_CORES):
        xp = xq_full[c * RPC : (c + 1) * RPC]
        xtile = np.zeros(((NSB + 1) * TPS * BLK, RPC), xcast)
        xtile[: NX + 2 * r] = xp.T
        # interleave 4 consecutive tiles side by side per superblock row-block
        xsb = (
            xtile.reshape(NSB + 1, TPS, BLK, RPC)
            .transpose(0, 2, 1, 3)
            .reshape((NSB + 1) * BLK, TPS * RPC)
        )
        in_maps.append({"xt": np.ascontiguousarray(xsb), **wmap})
    return in_maps


def assemble(results: list[dict]) -> np.ndarray:
    out = np.empty((NZ, NX), np.float32)
    for c in range(N_CORES):
        arr = np.asarray(results[c]["out"]).astype(np.float32)
        if MODE == "fp8o":
            arr /= np.float32(OUT_SCALE)
        cols = (
            arr.reshape(NCH // 4, BLK, 4, RPC)
            .transpose(0, 2, 1, 3)
            .reshape(NX, RPC)
        )
        out[c * RPC : (c + 1) * RPC] = cols.T
    return out


def kernel(feature, h_smooth) -> np.ndarray:
    from concourse.bass_utils import run_bass_kernel_spmd

    nc = build_nc()
    in_maps = make_in_maps(feature, h_smooth)
    res = run_bass_kernel_spmd(nc, in_maps, core_ids=list(range(N_CORES)))
    return assemble(res.results)


# revision 23
# speedup vs baseline: 1.1167x; 1.1167x over previous
"""Gaussian row-smoothing (sigma=h_smooth, truncate=4.0, reflect padding) on
8 Trainium2 NeuronCores.

Strategy
--------
Data-parallel over rows (nz=4096 -> 512 rows/core). The 1D conv along rows is
computed on the TensorEngine as a banded-Toeplitz matmul in the transposed
domain:

  host: per core, pad the [512, 8192] shard symmetrically by r=40 along cols,
        transpose to [8272, 512], cast to bf16 and lay out as 17 superblocks
        of 4 column-tiles [128, 512] each (tiles 64..67 mostly zero-padding;
        only tile 64 is ever transferred).

  device: output column-block b (128 cols x 512 rows, transposed layout) is
        psum_b = WA.T @ tile_b + WB.T @ tile_{b+1}
        where WA[p, j] = w[p - j]       (0 <= p-j <= 2r)
              WB[p, j] = w[128 + p - j] (0 <= 128+p-j <= 2r)
        are constant [128, 128] bf16 band matrices holding the 81-tap kernel.
        PSUM (f32) -> SBUF copy casts to bf16 (alternating DVE / ACT so
        neither engine becomes the bottleneck), output DMA'd as bf16.

  host: un-interleave, cast back to f32, transpose, concatenate.

All HBM traffic is bf16 (16.9 MB/core vs 33 MB for f32), which matters
because the kernel is HBM-bound: floor = 16.9 MB / 358 GB/s ~ 47 us.
Input loads ride the sync (SP) HWDGE ring as 512 KB fully-contiguous DMAs;
output stores ride the scalar (ACT) ring so their data dependencies never
stall input streaming. bf16 quantization of input/weights/output gives
l2 rel err ~3e-3 (tolerance 2e-2).
"""

import os
import numpy as np

NZ, NX = 4096, 8192
N_CORES = 8
RPC = NZ // N_CORES          # rows per core = 512
BLK = 128                    # column block (partition dim)
NCH = NX // BLK              # 64 output column blocks per row
NT = NCH + 1                 # 65 input tiles (one extra for the right overlap)
TPS = 4                      # tiles per input superblock (512 KB DMAs)
NSB = NT // TPS              # 16 full superblocks; tile 64 rides separately
GPO = 2                      # psum groups per output tile (4 blocks, 512 KB DMAs)
TRUNCATE = 4.0
# fp8o: e3m4 in+out; fp8: e3m4 in, bf16 out; bf16: bf16 in+out
MODE = os.environ.get("KERNEL_MODE", "fp8o")
OUT_SCALE = 16.0  # folded into the weights when the output is e3m4
X_BUFS = int(os.environ.get("KERNEL_XBUFS", "17"))
O_BUFS = int(os.environ.get("KERNEL_OBUFS", "4"))
COPY_SPLIT = os.environ.get("KERNEL_COPY_SPLIT", "1") == "1"
# Junk matmuls bridging the DMA-only prologue so the PE HAM clock gate
# (1.2 -> 2.4 GHz after ~3.4us sustained activity) lifts before real work.
N_WARMUP = int(os.environ.get("KERNEL_WARMUP", "12"))
OUT_ENG = os.environ.get("KERNEL_OUT_ENG", "gpsimd")
PREFETCH = os.environ.get("KERNEL_PREFETCH", "1") == "1"
W_ENG = os.environ.get("KERNEL_W_ENG", "sync")
SPLIT_LAST = os.environ.get("KERNEL_SPLIT_LAST", "1") == "1"


def OUT_DMA_ENG(nc):
    return {"gpsimd": nc.gpsimd, "scalar": nc.scalar, "sync": nc.sync}[OUT_ENG]

_NC_CACHE = {}


def _gauss_weights(sigma: float) -> tuple[np.ndarray, int]:
    radius = int(TRUNCATE * sigma + 0.5)
    x = np.arange(-radius, radius + 1, dtype=np.float32)
    w = np.exp(np.float32(-0.5) * (x / np.float32(sigma)) ** 2)
    w = w / np.sum(w)
    return w.astype(np.float32), radius


def _band_matrices(sigma: float) -> tuple[np.ndarray, np.ndarray, int]:
    w, r = _gauss_weights(sigma)
    ntaps = 2 * r + 1
    assert ntaps <= BLK, f"kernel supports radius <= 63, got {r}"
    wa = np.zeros((BLK, BLK), np.float32)
    wb = np.zeros((BLK, BLK), np.float32)
    p = np.arange(BLK)[:, None]
    j = np.arange(BLK)[None, :]
    k = p - j
    m = (k >= 0) & (k <= 2 * r)
    wa[m] = w[k[m]]
    k2 = k + BLK
    m2 = (k2 >= 0) & (k2 <= 2 * r)
    wb[m2] = w[k2[m2]]
    return wa, wb, r


def build_nc():
    """Build (and cache) the SPMD Bass program. Shapes are fixed; the band
    weights arrive as data, so one NEFF serves any h_smooth with radius<=63."""
    if "nc" in _NC_CACHE:
        return _NC_CACHE["nc"]
    import concourse.tile as tile
    from concourse import bacc, mybir

    f32 = mybir.dt.float32
    bf16 = mybir.dt.bfloat16
    xdt = bf16 if MODE == "bf16" else mybir.dt.float8e3
    odt = mybir.dt.float8e3 if MODE == "fp8o" else bf16

    nc = bacc.Bacc(None)
    # 17 superblocks of 4 tiles; last superblock only has tile 64 valid.
    xt = nc.declare_dram_parameter("xt", [(NSB + 1) * BLK, TPS * RPC], xdt,
                                   isOutput=False)
    wa_p = nc.declare_dram_parameter("wa", [BLK, BLK], bf16, isOutput=False)
    wb_p = nc.declare_dram_parameter("wb", [BLK, BLK], bf16, isOutput=False)
    # Output: 16 groups of 4 column-blocks, each [128, 4*512] contiguous.
    out = nc.declare_dram_parameter("out", [(NCH // 4) * BLK, 4 * RPC], odt,
                                    isOutput=True)

    with tile.TileContext(nc) as tc:
        with (
            tc.tile_pool(name="w", bufs=1) as wpool,
            tc.tile_pool(name="x", bufs=X_BUFS) as xpool,
            tc.tile_pool(name="xtl", bufs=1) as xtlpool,
            tc.tile_pool(name="ps", bufs=4, space="PSUM") as pspool,
            tc.tile_pool(name="o", bufs=O_BUFS) as opool,
        ):
            weng = OUT_DMA_ENG(nc) if W_ENG == "out" else {
                "gpsimd": nc.gpsimd, "scalar": nc.scalar, "sync": nc.sync
            }[W_ENG]
            wa_t = wpool.tile([BLK, BLK], bf16, tag="wa")
            weng.dma_start(wa_t[:], wa_p[:])
            wb_t = wpool.tile([BLK, BLK], bf16, tag="wb")
            weng.dma_start(wb_t[:], wb_p[:])

            if N_WARMUP:
                wu = pspool.tile([BLK, 2 * RPC], f32, tag="psum")
                for _ in range(N_WARMUP):
                    nc.tensor.matmul(
                        wu[:, 0:BLK], wa_t[:], wa_t[:], start=True, stop=True
                    )

            sb_bufs = {}

            def ensure_loaded(s):
                if s in sb_bufs:
                    return
                if s < NSB:
                    tl = xpool.tile([BLK, TPS * RPC], xdt, tag="xsb")
                    nc.sync.dma_start(tl[:], xt[s * BLK : (s + 1) * BLK, :])
                else:  # tail: only tile 64 (first slot of superblock 16)
                    tl = xtlpool.tile([BLK, RPC], xdt, tag="xtail")
                    nc.sync.dma_start(tl[:], xt[NSB * BLK : (NSB + 1) * BLK, 0:RPC])
                sb_bufs[s] = tl

            def tile_ap(t):
                s, slot = t // TPS, t % TPS
                ensure_loaded(s)
                if s < NSB:
                    return sb_bufs[s][:, slot * RPC : (slot + 1) * RPC]
                return sb_bufs[s][:]

            otile_box = [None]

            def emit_output(g, ps):
                h = g % GPO
                if h == 0:
                    otile_box[0] = opool.tile(
                        [BLK, GPO * 2 * RPC], odt, tag="otile",
                        name=f"ot{g // GPO}",
                    )
                ot = otile_box[0]
                dst = ot[:, h * 2 * RPC : (h + 1) * 2 * RPC]
                if COPY_SPLIT:
                    # halve group-copy latency: DVE takes bank A, ACT bank B
                    nc.vector.tensor_copy(dst[:, 0:RPC], ps[:, 0:RPC])
                    nc.scalar.copy(dst[:, RPC:], ps[:, RPC:])
                else:
                    nc.vector.tensor_copy(dst, ps[:])
                if h == GPO - 1:
                    g4 = g // GPO
                    if SPLIT_LAST and g == NCH // 2 - 1:
                        # last otile: ship each half right after its copy so
                        # the final chunk doesn't wait for both copies
                        OUT_DMA_ENG(nc).dma_start(
                            out[g4 * BLK : (g4 + 1) * BLK, 0 : 2 * RPC],
                            ot[:, 0 : 2 * RPC],
                        )
                        OUT_DMA_ENG(nc).dma_start(
                            out[g4 * BLK : (g4 + 1) * BLK, 2 * RPC :],
                            ot[:, 2 * RPC :],
                        )
                    else:
                        OUT_DMA_ENG(nc).dma_start(
                            out[g4 * BLK : (g4 + 1) * BLK, :], ot[:]
                        )

            if PREFETCH:
                # queue every input DMA ahead of any output DMA so an
                # output's data-dependency stall can never delay an input
                # when both share the sync HWDGE ring
                assert X_BUFS >= NSB + 1
                for s in range(NSB + 1):
                    ensure_loaded(s)

            WAVE = 4  # psum groups per wave (all 8 PSUM banks)
            for wv in range(NCH // 2 // WAVE):
                gs = [WAVE * wv + i for i in range(WAVE)]
                pss = [
                    pspool.tile([BLK, 2 * RPC], f32, tag="psum", name=f"ps{g}")
                    for g in gs
                ]
                # one LDWEIGHTS per pass instead of per matmul: all wa
                # accumulations for the wave, then all wb
                for i, g in enumerate(gs):
                    nc.tensor.matmul(pss[i][:, 0:RPC], wa_t[:], tile_ap(2 * g),
                                     start=True, stop=False)
                    nc.tensor.matmul(pss[i][:, RPC:], wa_t[:], tile_ap(2 * g + 1),
                                     start=True, stop=False)
                for i, g in enumerate(gs):
                    nc.tensor.matmul(pss[i][:, 0:RPC], wb_t[:], tile_ap(2 * g + 1),
                                     start=False, stop=True)
                    nc.tensor.matmul(pss[i][:, RPC:], wb_t[:], tile_ap(2 * g + 2),
                                     start=False, stop=True)
                for i, g in enumerate(gs):
                    emit_output(g, pss[i])

    nc.finalize()
    _NC_CACHE["nc"] = nc
    return nc


def _shaped_quant_e3m4(a: np.ndarray):
    """Cast rows to float8_e3m4 with first-order error feedback along the row.
    The Gaussian filter is a strong low-pass, so pushing quantization noise
    to high frequencies makes it vanish from the output (~14x less noise
    than round-to-nearest while sending the identical byte count)."""
    import ml_dtypes

    q = np.empty(a.shape, ml_dtypes.float8_e3m4)
    e = np.zeros(a.shape[0], np.float32)
    for j in range(a.shape[1]):
        v = a[:, j] + e
        qj = v.astype(ml_dtypes.float8_e3m4)
        q[:, j] = qj
        e = v - qj.astype(np.float32)
    return q


def make_in_maps(feature: np.ndarray, h_smooth) -> list[dict]:
    import ml_dtypes

    sigma = float(int(h_smooth))
    wa, wb, r = _band_matrices(sigma)
    ws = np.float32(OUT_SCALE) if MODE == "fp8o" else np.float32(1.0)
    wmap = {
        "wa": (wa * ws).astype(ml_dtypes.bfloat16),
        "wb": (wb * ws).astype(ml_dtypes.bfloat16),
    }
    feature = np.asarray(feature, dtype=np.float32)
    assert feature.shape == (NZ, NX)
    xp_full = np.pad(feature, ((0, 0), (r, r)), mode="symmetric")  # [nz, nx+2r]
    if MODE != "bf16":
        xq_full = _shaped_quant_e3m4(xp_full)
        xcast = ml_dtypes.float8_e3m4
    else:
        xq_full = xp_full.astype(ml_dtypes.bfloat16)
        xcast = ml_dtypes.bfloat16
    in_maps = []
    for c in range(N_CORES):
        xp = xq_full[c * RPC : (c + 1) * RPC]
        xtile = np.zeros(((NSB + 1) * TPS * BLK, RPC), xcast)
        xtile[: NX + 2 * r] = xp.T
        # interleave 4 consecutive tiles side by side per superblock row-block
        xsb = (
            xtile.reshape(NSB + 1, TPS, BLK, RPC)
            .transpose(0, 2, 1, 3)
            .reshape((NSB + 1) * BLK, TPS * RPC)
        )
        in_maps.append({"xt": np.ascontiguousarray(xsb), **wmap})
    return in_maps


def assemble(results: list[dict]) -> np.ndarray:
    out = np.empty((NZ, NX), np.float32)
    for c in range(N_CORES):
        arr = np.asarray(results[c]["out"]).astype(np.float32)
        if MODE == "fp8o":
            arr /= np.float32(OUT_SCALE)
        cols = (
            arr.reshape(NCH // 4, BLK, 4, RPC)
            .transpose(0, 2, 1, 3)
            .reshape(NX, RPC)
        )
        out[c * RPC : (c + 1) * RPC] = cols.T
    return out


def kernel(feature, h_smooth) -> np.ndarray:
    from concourse.bass_utils import run_bass_kernel_spmd

    nc = build_nc()
    in_maps = make_in_maps(feature, h_smooth)
    res = run_bass_kernel_spmd(nc, in_maps, core_ids=list(range(N_CORES)))
    return assemble(res.results)
